# revision 48
# baseline (speedup 1.0000x reference)
"""ChildSumTreeLSTM (N=8192 complete 8-ary tree) on 8 TRN2 NeuronCores.

Decomposition (all tree structure is compile-time static):
- nodes 0..1023 are internal (children of p = 8p+1..8p+8), 1024..8191 leaves.
- Phase A (per core): iou_x/fx_x projections for the ~1096 node-columns this
  core owns, feature-major, fp16 matmuls on the PE (f32 PSUM accumulate).
  Leaf (h, c) states are written straight into persistent SBUF tiles
  (L4*/L3*); parent projections land in node-major SBUF tiles (nm*).
- 5 sequential rounds of internal levels: R4 (parents 585..1023, 439),
  R3 (73..584, 512), R2 (9..72, 64), R1 (1..8, 8), R0 (root).
  Each round is node-sharded across the 8 cores so that every child a core
  needs was computed locally, except: R4 results are AllGather'ed (core 0
  consumes them for R3), and R1 results are AllGather'ed (every core then
  computes the root; core 0's answer is returned).
"""
import sys
import functools

sys.path.insert(0, '/opt/trn_rl_repo')

import numpy as np
import concourse.bacc as bacc
import concourse.mybir as mybir
import concourse.tile as tile
from concourse.bass_utils import run_bass_kernel_spmd

DT = mybir.dt
AF = mybir.ActivationFunctionType
F16 = DT.float16

NCORES = 8
N = 8192
M = 1024
C4 = [54, 54, 55, 55, 55, 55, 55, 56]
S4 = [585, 639, 693, 748, 803, 858, 913, 968]
NB = [56, 64, 8, 1, 1]          # parents per round (uniform per core)
NCOLS = 1096                    # 448 (R4 children) + 512 (R3 children) + 136 parents
QW = 274                        # quarter-block width (4 quarters)


def _core_cols(i):
    # col order: [136 parents][448 R4 children][512 R3 children] so the
    # parent projections (-> nm tables) complete in quarter 0.
    cols = []
    for q in range(56):
        cols.append(S4[i] + q if q < C4[i] else -1)
    cols += [73 + 64 * i + j for j in range(64)]
    cols += [9 + 8 * i + j for j in range(8)]
    cols += [1 + i, 0] + [-1] * 6
    for pl in range(56):
        for k in range(8):
            if pl < C4[i]:
                node = 8 * (S4[i] + pl) + 1 + k
                cols.append(node if node < N else -1)
            else:
                cols.append(-1)
    for b in range(512):
        node = 585 + 512 * i + b
        cols.append(-1 if (i == 0 and b < 439) else node)
    return cols


@functools.lru_cache(maxsize=1)
def _build():
    nc = bacc.Bacc(trn_type="TRN2", target_bir_lowering=False, debug=False,
                   num_devices=NCORES)

    xT_d = nc.dram_tensor("xT", [4, 128, 8 * QW], F16, kind="ExternalInput")
    WAG_d = nc.dram_tensor("WAG", [8, 128, 4096], F16, kind="ExternalInput")
    WRG_d = nc.dram_tensor("WRG", [8, 128, 4096], F16, kind="ExternalInput")
    BT_d = nc.dram_tensor("BT", [128, 32], DT.float32, kind="ExternalInput")
    SEL_d = nc.dram_tensor("SEL", [128, 1024], F16, kind="ExternalInput")
    I_d = nc.dram_tensor("I128", [128, 128], F16, kind="ExternalInput")
    I4_d = nc.dram_tensor("I4", [128, 32], F16, kind="ExternalInput")
    rh_d = nc.dram_tensor("root_h", [1, M], DT.float32, kind="ExternalOutput")
    rc_d = nc.dram_tensor("root_c", [1, M], DT.float32, kind="ExternalOutput")

    RG = [list(range(NCORES))]

    with tile.TileContext(nc) as tc:
        with (
            tc.tile_pool(name="dram", bufs=1, space="DRAM") as dram,
            tc.tile_pool(name="persist", bufs=1) as pp,
            tc.tile_pool(name="wpool", bufs=1) as wp,
            tc.tile_pool(name="leafp", bufs=1) as lp,
            tc.tile_pool(name="nmp", bufs=1) as nmp,
        ):
            nm_dram = dram.tile([136, 4096], F16)
            spill3 = dram.tile([2, 128, 8, 512], F16)
            dum_in = dram.tile([128, 16], F16)
            dum_out = dram.tile([NCORES, 128, 16], F16, addr_space="Shared")
            ag_in = dram.tile([2, 128, 8, 56], F16)
            ag_out = dram.tile([NCORES, 2, 128, 8, 56], F16,
                               addr_space="Shared")
            agb_in = dram.tile([2, 128, 8, 1], F16)
            agb_out = dram.tile([NCORES, 2, 128, 8, 1], F16,
                                addr_space="Shared")

            I_t = pp.tile([128, 128], F16)
            I4_t = pp.tile([128, 32], F16)
            SEL_t = pp.tile([64, 512], F16)
            BT_t = pp.tile([128, 32], DT.float32)

            # persistent R4-leaf states (feature-major: [p, jm, col]);
            # R3 leaf states spill to DRAM (readback hides under AllGather)
            L4c = lp.tile([128, 8, 448], F16, name="L4c")
            L4h = lp.tile([128, 8, 448], F16, name="L4h")

            # node-major parent projections, preloaded from nm_dram right
            # after Phase A. nm210 packs R2 rows 0-7, R1 row 8, R0 row 9.
            nm4 = nmp.tile([56, 4096], F16, name="nm4")
            nm3 = nmp.tile([64, 4096], F16, name="nm3")
            nm210 = nmp.tile([10, 4096], F16, name="nm210")
            # feature-major fx (+bias) for the 10 R2/R1/R0 parents
            pcolF = pp.tile([128, 8, 10], DT.float32, name="pcolF")

            # weight chunks: WAg[jm][p, k, gi*128+c] (Phase A), WRg[ch] (rounds)
            WAg = [wp.tile([128, 8, 512], F16, tag=f"wa{j}", bufs=1,
                           name=f"WAg{j}") for j in range(8)]
            WRg = [wp.tile([128, 8, 512], F16, tag=f"wr{j}", bufs=1,
                           name=f"WRg{j}") for j in range(8)]

            # ---------------- Phase A + leaves ----------------
            with (
                tc.tile_pool(name="xp", bufs=1) as xp,
                tc.tile_pool(name="pap", bufs=1, space="PSUM") as pap,
                tc.tile_pool(name="drp", bufs=1) as drp,
            ):
                xsq = [xp.tile([128, 8, QW], F16, tag=f"xs{q}", bufs=1,
                               name=f"xsq{q}") for q in range(4)]
                # small constants on the scalar HWDGE ring (parallel with sync)
                nc.scalar.dma_start(BT_t[:], BT_d[:])
                nc.scalar.dma_start(I_t[:], I_d[:])
                nc.scalar.dma_start(SEL_t[:], SEL_d[0:64, 0:512])
                nc.scalar.dma_start(I4_t[:], I4_d[:])
                # dummy collective: absorb collective first-call latency and
                # launch skew while Phase A's DMA loads stream
                nc.gpsimd.collective_compute(
                    "AllGather", mybir.AluOpType.bypass, replica_groups=RG,
                    ins=[dum_in.opt()], outs=[dum_out.opt()])
                # bulk loads on the sync HWDGE ring in exact use order
                nc.sync.dma_start(
                    xsq[0][:], xT_d[0].rearrange("p (k w) -> p k w", k=8))
                for j in range(8):
                    nc.sync.dma_start(
                        WAg[j][:], WAG_d[j].rearrange("p (k c) -> p k c", k=8))
                for q in range(1, 4):
                    nc.sync.dma_start(
                        xsq[q][:], xT_d[q].rearrange("p (k w) -> p k w", k=8))
                for j in range(8):
                    nc.sync.dma_start(
                        WRg[j][:], WRG_d[j].rearrange("p (k c) -> p k c", k=8))

                def drain_ranges(b4):
                    """Leaf col ranges of quarter b4: global cols [136, 584)
                    are L4 (idx g-136), [584, 1096) spill to L3 (idx g-584).
                    Yields (is_l4, gs, ge, ls, le): dst idx range + local
                    [ls, le) range within the quarter."""
                    cb = QW * b4
                    a = cb + (136 if b4 == 0 else 0)
                    b = cb + QW
                    if a < 584:
                        e = min(b, 584)
                        yield (True, a - 136, e - 136, a - cb, e - cb)
                    if b > 584:
                        a2 = max(a, 584)
                        yield (False, a2 - 584, b - 584, a2 - cb, b - cb)

                for b4 in range(4):
                    for jm in range(8):
                        ps = {}
                        for gi in range(3):
                            ps[gi] = pap.tile([128, QW], DT.float32, tag="pa",
                                              bufs=6, name=f"pa_{jm}_{b4}_{gi}")
                        psf = None
                        if b4 == 0:
                            psf = pap.tile([128, 136], DT.float32, tag="pa",
                                           bufs=6, name=f"paf_{jm}")
                        for k in range(8):
                            for gi in range(3):
                                nc.tensor.matmul(
                                    ps[gi][:],
                                    WAg[jm][:, k, 128 * gi:128 * (gi + 1)],
                                    xsq[b4][:, k, :],
                                    start=(k == 0), stop=(k == 7))
                            if b4 == 0:
                                nc.tensor.matmul(
                                    psf[:],
                                    WAg[jm][:, k, 384:512],
                                    xsq[0][:, k, 0:136],
                                    start=(k == 0), stop=(k == 7))
                        # leaf elementwise drains -> straight into SBUF state
                        ls0 = 136 if b4 == 0 else 0
                        si = drp.tile([128, QW], F16, tag="dr", bufs=8,
                                      name=f"si_{jm}_{b4}")
                        tu = drp.tile([128, QW], F16, tag="dr", bufs=8,
                                      name=f"tu_{jm}_{b4}")
                        nc.scalar.activation(si[:, ls0:QW], ps[0][:, ls0:QW],
                                             AF.Sigmoid,
                                             bias=BT_t[:, jm:jm + 1])
                        nc.scalar.activation(tu[:, ls0:QW], ps[2][:, ls0:QW],
                                             AF.Tanh,
                                             bias=BT_t[:, jm + 16:jm + 17])
                        ct3 = None
                        for is4, gs, ge, ls, le in drain_ranges(b4):
                            if is4:
                                nc.vector.tensor_mul(L4c[:, jm, gs:ge],
                                                     si[:, ls:le], tu[:, ls:le])
                            else:
                                ct3 = drp.tile([128, QW], F16, tag="c3",
                                               bufs=8, name=f"ct3_{jm}_{b4}")
                                nc.vector.tensor_mul(ct3[:, ls:le],
                                                     si[:, ls:le], tu[:, ls:le])
                                nc.scalar.dma_start(spill3[0, :, jm, gs:ge],
                                                    ct3[:, ls:le])
                        so = drp.tile([128, QW], F16, tag="dr", bufs=8,
                                      name=f"so_{jm}_{b4}")
                        nc.scalar.activation(so[:, ls0:QW], ps[1][:, ls0:QW],
                                             AF.Sigmoid,
                                             bias=BT_t[:, jm + 8:jm + 9])
                        tanc = drp.tile([128, QW], F16, tag="dr", bufs=8,
                                        name=f"tanc_{jm}_{b4}")
                        for is4, gs, ge, ls, le in drain_ranges(b4):
                            if is4:
                                nc.scalar.activation(tanc[:, ls:le],
                                                     L4c[:, jm, gs:ge], AF.Tanh)
                                nc.vector.tensor_mul(L4h[:, jm, gs:ge],
                                                     so[:, ls:le], tanc[:, ls:le])
                            else:
                                nc.scalar.activation(tanc[:, ls:le],
                                                     ct3[:, ls:le], AF.Tanh)
                                ht3 = drp.tile([128, QW], F16, tag="c3",
                                               bufs=8, name=f"ht3_{jm}_{b4}")
                                nc.vector.tensor_mul(ht3[:, ls:le],
                                                     so[:, ls:le], tanc[:, ls:le])
                                nc.sync.dma_start(spill3[1, :, jm, gs:ge],
                                                  ht3[:, ls:le])
                        # parent drains + transpose to node-major (cols
                        # 0..136 of quarter 0 = parent cols 0..135)
                        if b4 == 0:
                            for gi in range(4):
                                j = jm + 8 * gi
                                bcol = j if gi < 3 else 24 + jm
                                src = (ps[gi][:, 0:136] if gi < 3
                                       else psf[:, 0:136])
                                fm = drp.tile([128, 136], F16, tag="fm",
                                              bufs=4, name=f"fm_{jm}_{gi}")
                                nc.scalar.activation(
                                    fm[:, 0:136], src, AF.Identity,
                                    bias=BT_t[:, bcol:bcol + 1])
                                for half in range(2):
                                    qn = 128 if half == 0 else 8
                                    tp = pap.tile([128, 136], F16,
                                                  tag="tp", bufs=2,
                                                  name=f"tp_{jm}_{gi}_{half}")
                                    nc.tensor.transpose(
                                        tp[0:qn, 0:128],
                                        fm[:, 128 * half:128 * half + qn],
                                        I_t[:, :])
                                    tsb = drp.tile([128, 136], F16,
                                                   tag="tsb", bufs=4,
                                                   name=f"tsb_{jm}_{gi}_{half}")
                                    nc.vector.tensor_copy(tsb[0:qn, 0:128],
                                                          tp[0:qn, 0:128])
                                    nc.sync.dma_start(
                                        nm_dram[128 * half:128 * half + qn,
                                                128 * j:128 * (j + 1)],
                                        tsb[0:qn, 0:128])
                                if gi == 3:
                                    nc.vector.tensor_copy(
                                        pcolF[:, jm, 0:10], fm[:, 120:130])
                    if b4 == 0:
                        # preload node-major per-round parent tables into
                        # SBUF as soon as the quarter-0 nm writes land.
                        # MUST be on the sync ring: the scalar ring is FIFO
                        # with activations, and these block on the nm writes.
                        nc.sync.dma_start(nm4[0:56, :], nm_dram[0:56, :])
                        nc.sync.dma_start(nm3[0:64, :], nm_dram[56:120, :])
                        nc.sync.dma_start(nm210[0:10, :],
                                          nm_dram[120:130, :])
                # zero the single real pad-child column (local col 447:
                # node 1023's 8th child on core 7; harmless on other cores)
                nc.vector.memset(L4c[:, :, 447:448], 0.0)
                nc.vector.memset(L4h[:, :, 447:448], 0.0)

            # ---------------- Rounds ----------------
            import os as _os
            _SKIP_ROUNDS = bool(_os.environ.get('PHASE_A_ONLY'))
            _UPTO = int(_os.environ.get('ROUNDS_UPTO', '99'))
            if _SKIP_ROUNDS:
                dum = pp.tile([128, 8], DT.float32, name="dum")
                nc.vector.memset(dum[:], 0.0)
                nc.sync.dma_start(
                    rc_d[0, :].rearrange("(m p) -> p m", p=128), dum[:])
                nc.sync.dma_start(
                    rh_d[0, :].rearrange("(m p) -> p m", p=128), dum[:])
            if not _SKIP_ROUNDS:
              with (
                  tc.tile_pool(name="rps", bufs=1, space="PSUM") as rps,
                  tc.tile_pool(name="chp", bufs=1) as chp,
                  tc.tile_pool(name="rwp", bufs=1) as rwp,
                  tc.tile_pool(name="sink", bufs=1) as sink,
              ):
                  ch3c = chp.tile([128, 8, 512], F16, name="ch3c")
                  ch3h = chp.tile([128, 8, 512], F16, name="ch3h")
                  c3_c = sink.tile([128, 8, 64], F16)
                  c3_h = sink.tile([128, 8, 64], F16)
                  c2_c = sink.tile([128, 8, 8], F16)
                  c2_h = sink.tile([128, 8, 8], F16)
                  c1_c = sink.tile([128, 8, 8], F16)
                  c1_h = sink.tile([128, 8, 8], F16)
                  st4_c = sink.tile([128, 8, 56], F16)
                  st4_h = sink.tile([128, 8, 56], F16)
                  st1_c = sink.tile([128, 8, 1], F16)
                  st1_h = sink.tile([128, 8, 1], F16)
                  c1raw = sink.tile([8, 2, 128, 8], F16)
                  rootc_sb = sink.tile([128, 8], F16)
                  rooth_sb = sink.tile([128, 8], F16)
                  rootc_f32 = sink.tile([128, 8], DT.float32)
                  rooth_f32 = sink.tile([128, 8], DT.float32)

                  # (tile, selector-row offset, selector row count)
                  NM = [(nm4, 0, 56), (nm3, 0, 64), (nm210, 0, 10),
                        (nm210, 8, 10), (nm210, 9, 10)]

                  def group8_sum(prod_ap, out_ap, nb, rn, jm):
                      """out[p, n] = sum_k prod[p, 8n + k]."""
                      a = prod_ap.rearrange("p (n k) -> p n k", k=8)
                      l1 = rwp.tile([128, 256], F16, tag="lvl1", bufs=2,
                                    name=f"l1_{rn}_{jm}")
                      l1v = l1[:, 0:nb * 4].rearrange("p (n k) -> p n k", k=4)
                      nc.vector.tensor_add(l1v, a[:, :, 0:4], a[:, :, 4:8])
                      l2 = rwp.tile([128, 128], F16, tag="lvl2", bufs=2,
                                    name=f"l2_{rn}_{jm}")
                      l2v = l2[:, 0:nb * 2].rearrange("p (n k) -> p n k", k=2)
                      nc.vector.tensor_add(l2v, l1v[:, :, 0:2], l1v[:, :, 2:4])
                      # out[p, n] = l2[p, 2n] + l2[p, 2n+1]  (stride-2 views)
                      e0 = l2v[:, :, 0:1].rearrange("p n k -> p (n k)")
                      e1 = l2v[:, :, 1:2].rearrange("p n k -> p (n k)")
                      nc.vector.tensor_add(out_ap, e0, e1)

                  def iou_small(rn, nb, nm_t, r0, nrows, csumT):
                      """Col-tiled iou for nb<=8: each ch gets its own PSUM
                      bank; 4-way array col-group concurrency."""
                      ipsc = [rps.tile([128, 512], DT.float32, tag="iou",
                                       bufs=6, name=f"ipsS_{rn}_{c}")
                              for c in range(6)]
                      for k in range(8):
                          for ch in range(6):
                              g = 32 * (ch % 4)
                              nc.tensor.matmul(
                                  ipsc[ch][g:g + nb, :],
                                  csumT[:, k, 0:nb],
                                  WRg[ch][:, k, :],
                                  start=(k == 0), stop=False,
                                  tile_position=(0, g))
                      for ch in range(6):
                          g = 32 * (ch % 4)
                          nc.tensor.matmul(
                              ipsc[ch][g:g + nb, :],
                              I_t[0:nrows, r0:r0 + nb],
                              nm_t[0:nrows, 512 * ch:512 * (ch + 1)],
                              start=False, stop=True,
                              tile_position=(0, g))
                      return ipsc

                  def iou_full(rn, nb, nm_t, r0, nrows, csumT):
                      """Node-major iou, one PSUM bank per ch."""
                      ipsc = [rps.tile([64, 512], DT.float32, tag="iou",
                                       bufs=6, name=f"ips_{rn}_{c}")
                              for c in range(6)]
                      for k in range(8):
                          for ch in range(6):
                              nc.tensor.matmul(
                                  ipsc[ch][0:nb, :],
                                  csumT[:, k, 0:nb],
                                  WRg[ch][:, k, :],
                                  start=(k == 0), stop=False)
                      for ch in range(6):
                          nc.tensor.matmul(
                              ipsc[ch][0:nb, :],
                              I_t[0:nrows, r0:r0 + nb],
                              nm_t[0:nrows, 512 * ch:512 * (ch + 1)],
                              start=False, stop=True)
                      return ipsc

                  def run_round(rn, get_chC, get_chH, out_c, out_h):
                      nb = NB[rn]
                      nm_t, r0, nrows = NM[rn]
                      w8 = 8 * nb
                      small = nb <= 8
                      # 1. csum (feature-major)
                      csumT = rwp.tile([128, 8, 64], F16, tag="csum",
                                       bufs=1, name=f"csum_{rn}")
                      for m in range(8):
                          group8_sum(get_chC(m), csumT[:, m, 0:nb], nb, rn, m)
                      # 2. iou
                      if small:
                          ipsc = iou_small(rn, nb, nm_t, r0, nrows, csumT)
                      else:
                          ipsc = iou_full(rn, nb, nm_t, r0, nrows, csumT)
                      # 3-5. f gates (feature-major), prod, fc
                      fcT = rwp.tile([128, 8, 64], F16, tag="fcT", bufs=1,
                                     name=f"fcT_{rn}")
                      use_bias_fx = nb == 1
                      for j in range(8):
                          fps = rps.tile([128, 512], DT.float32, tag="fp", bufs=2,
                                         name=f"fps_{rn}_{j}")
                          for k in range(8):
                              nc.tensor.matmul(
                                  fps[:, 0:w8],
                                  WRg[6 + j // 4][:, k,
                                                  128 * (j % 4):128 * (j % 4 + 1)],
                                  get_chC(k)[:, 0:w8],
                                  start=(k == 0),
                                  stop=(use_bias_fx and k == 7))
                          if not use_bias_fx:
                              nc.tensor.matmul(
                                  fps[:, 0:w8],
                                  nm_t[r0:r0 + nb,
                                       3072 + 128 * j:3072 + 128 * (j + 1)],
                                  SEL_t[0:nb, 0:w8],
                                  start=False, stop=True)
                          fsb = rwp.tile([128, 512], F16, tag="fsb", bufs=2,
                                         name=f"fsb_{rn}_{j}")
                          if use_bias_fx:
                              # fx (+b) add via per-partition activation bias
                              nc.scalar.activation(
                                  fsb[:, 0:w8], fps[:, 0:w8], AF.Sigmoid,
                                  bias=pcolF[:, j, r0:r0 + 1])
                          else:
                              nc.scalar.activation(fsb[:, 0:w8], fps[:, 0:w8],
                                                   AF.Sigmoid)
                          prod = rwp.tile([128, 512], F16, tag="fsb", bufs=2,
                                          name=f"prod_{rn}_{j}")
                          nc.vector.tensor_mul(prod[:, 0:w8], fsb[:, 0:w8],
                                               get_chH(j)[:, 0:w8])
                          group8_sum(prod[:, 0:w8], fcT[:, j, 0:nb], nb, rn, 100 + j)
                      # 6. gates from iou psum
                      tw = max(2, nb)
                      if small:
                          # ch c lives at partitions 32*(c%4).. of its bank
                          gio = rwp.tile([128, 512], F16, tag="g", bufs=3,
                                         name=f"gio_{rn}")
                          for c in range(4):
                              g = 32 * c
                              nc.scalar.activation(gio[g:g + nb, :],
                                                   ipsc[c][g:g + nb, :],
                                                   AF.Sigmoid)
                          gu = rwp.tile([64, 512], F16, tag="g", bufs=3,
                                        name=f"gu_{rn}")
                          for c in range(2):
                              g = 32 * c
                              nc.scalar.activation(gu[g:g + nb, :],
                                                   ipsc[4 + c][g:g + nb, :],
                                                   AF.Tanh)
                          p1 = rwp.tile([64, 512], F16, tag="g", bufs=3,
                                        name=f"p1_{rn}")
                          nc.vector.tensor_mul(p1[0:64, :], gio[0:64, :],
                                               gu[0:64, :])

                          def tsrc(which, m):
                              # (tile, row base) of feature chunk m
                              if which == 'p1':
                                  return p1, 32 * (m // 4)
                              return gio, 64 + 32 * (m // 4)
                      else:
                          si = rwp.tile([64, 1024], F16, tag="g", bufs=3,
                                        name=f"si_{rn}")
                          tu = rwp.tile([64, 1024], F16, tag="g", bufs=3,
                                        name=f"tu_{rn}")
                          for c in range(2):
                              nc.scalar.activation(si[0:nb, 512 * c:512 * (c + 1)],
                                                   ipsc[c][0:nb, :], AF.Sigmoid)
                              nc.scalar.activation(tu[0:nb, 512 * c:512 * (c + 1)],
                                                   ipsc[4 + c][0:nb, :], AF.Tanh)
                          p1 = rwp.tile([64, 1024], F16, tag="g", bufs=3,
                                        name=f"p1_{rn}")
                          nc.vector.tensor_mul(p1[0:nb, :], si[0:nb, :],
                                               tu[0:nb, :])
                          so = rwp.tile([64, 1024], F16, tag="g", bufs=3,
                                        name=f"so_{rn}")
                          for c in range(2):
                              nc.scalar.activation(so[0:nb, 512 * c:512 * (c + 1)],
                                                   ipsc[2 + c][0:nb, :],
                                                   AF.Sigmoid)

                          def tsrc(which, m):
                              if which == 'p1':
                                  return p1, 0
                              return so, 0
                      # 7-8. transpose to feature-major, combine
                      for m in range(8):
                          if small:
                              mm = 128 * (m % 4)
                          else:
                              mm = 128 * m
                          t1, b1 = tsrc('p1', m)
                          idm = I_t if b1 == 0 else I4_t
                          tp1 = rps.tile([128, 64], F16, tag="fp", bufs=2,
                                         name=f"tp1_{rn}_{m}")
                          nc.tensor.transpose(tp1[:, 0:tw],
                                              t1[b1:b1 + nb, mm:mm + 128],
                                              idm[b1:b1 + nb, 0:tw],
                                              tile_position=(b1, 0))
                          cm = out_c(m)
                          nc.vector.tensor_add(cm, tp1[:, 0:nb], fcT[:, m, 0:nb])
                          t2, b2 = tsrc('so', m)
                          idm = I_t if b2 == 0 else I4_t
                          tso = rps.tile([128, 64], F16, tag="fp", bufs=2,
                                         name=f"tso_{rn}_{m}")
                          nc.tensor.transpose(tso[:, 0:tw],
                                              t2[b2:b2 + nb, mm:mm + 128],
                                              idm[b2:b2 + nb, 0:tw],
                                              tile_position=(b2, 0))
                          tanc = rwp.tile([128, 64], F16, tag="tanc", bufs=2,
                                          name=f"tanc_{rn}_{m}")
                          nc.scalar.activation(tanc[:, 0:nb], cm, AF.Tanh)
                          nc.vector.tensor_mul(out_h(m), tso[:, 0:nb],
                                               tanc[:, 0:nb])

                  # ---- R4 ----
                  if _UPTO >= 1:
                    run_round(0,
                            lambda m: L4c[:, m, 0:448],
                            lambda m: L4h[:, m, 0:448],
                            lambda m: st4_c[:, m, 0:56],
                            lambda m: st4_h[:, m, 0:56])
                  if _UPTO >= 2:
                    nc.sync.dma_start(ag_in[0], st4_c[:])
                    nc.sync.dma_start(ag_in[1], st4_h[:])
                    nc.gpsimd.collective_compute(
                        "AllGather", mybir.AluOpType.bypass, replica_groups=RG,
                        ins=[ag_in.opt()], outs=[ag_out.opt()])
                  # ---- R3 ----
                  if _UPTO >= 3:
                    nc.sync.dma_start(ch3c[:], spill3[0])
                    nc.scalar.dma_start(ch3h[:], spill3[1])
                    pid = nc.gpsimd.partition_id()
                    with tc.If(pid == 0):
                        for r in range(NCORES):
                            off = S4[r] - 585
                            nc.gpsimd.dma_start(
                                ch3c[:, :, off:off + C4[r]],
                                ag_out[r, 0, :, :, 0:C4[r]])
                            nc.gpsimd.dma_start(
                                ch3h[:, :, off:off + C4[r]],
                                ag_out[r, 1, :, :, 0:C4[r]])
                    run_round(1,
                            lambda m: ch3c[:, m, 0:512],
                            lambda m: ch3h[:, m, 0:512],
                            lambda m: c3_c[:, m, 0:64],
                            lambda m: c3_h[:, m, 0:64])
                  # ---- R2 ----
                  if _UPTO >= 3:
                    run_round(2,
                            lambda m: c3_c[:, m, :], lambda m: c3_h[:, m, :],
                            lambda m: c2_c[:, m, 0:8],
                            lambda m: c2_h[:, m, 0:8])
                  # ---- R1 ----
                  if _UPTO >= 4:
                    run_round(3,
                            lambda m: c2_c[:, m, :], lambda m: c2_h[:, m, :],
                            lambda m: st1_c[:, m, 0:1],
                            lambda m: st1_h[:, m, 0:1])
                  if _UPTO >= 5:
                    nc.sync.dma_start(agb_in[0], st1_c[:])
                    nc.sync.dma_start(agb_in[1], st1_h[:])
                    nc.gpsimd.collective_compute(
                        "AllGather", mybir.AluOpType.bypass, replica_groups=RG,
                        ins=[agb_in.opt()], outs=[agb_out.opt()])
                    nc.sync.dma_start(
                        c1raw[:], agb_out.rearrange("r s p k one -> r s p (k one)"))
                    for state in range(2):
                        dst = c1_c if state == 0 else c1_h
                        for k in range(8):
                            tpk = rps.tile([128, 8], F16, tag="fp", bufs=2,
                                           name=f"tpk_{state}_{k}")
                            nc.tensor.transpose(tpk[:, 0:8],
                                                c1raw[0:8, state, :, k],
                                                I_t[0:8, 0:8])
                            nc.vector.tensor_copy(dst[:, k, 0:8], tpk[:, 0:8])
                  # ---- R0 ----
                  if _UPTO >= 6:
                    run_round(4,
                            lambda m: c1_c[:, m, :], lambda m: c1_h[:, m, :],
                            lambda m: rootc_sb[:, m:m + 1],
                            lambda m: rooth_sb[:, m:m + 1])
                  if _UPTO < 6:
                      nc.vector.memset(rootc_sb[:], 0.0)
                      nc.vector.memset(rooth_sb[:], 0.0)
                  nc.vector.tensor_copy(rootc_f32[:], rootc_sb[:])
                  nc.vector.tensor_copy(rooth_f32[:], rooth_sb[:])
                  nc.sync.dma_start(
                      rc_d[0, :].rearrange("(m p) -> p m", p=128), rootc_f32[:])
                  nc.sync.dma_start(
                      rh_d[0, :].rearrange("(m p) -> p m", p=128), rooth_f32[:])

    nc.compile()
    return nc


def _preprocess(inputs, children, w_ioux, b_ioux, w_iouh, b_iouh,
                w_fx, b_fx, w_fh, b_fh):
    f32 = np.float32
    f16 = np.float16
    inputs = np.ascontiguousarray(inputs, dtype=f32)
    b_tot = (np.asarray(b_ioux) + np.asarray(b_iouh)).astype(f32)
    b_fhx = (np.asarray(b_fx) + np.asarray(b_fh)).astype(f32)

    X = inputs.T                                           # [1024, 8192]
    Wcat = np.concatenate([np.asarray(w_ioux, dtype=f32),
                           np.asarray(w_fx, dtype=f32)], axis=0)   # [4096, 1024]
    WcatT = Wcat.T.astype(f16)                             # [1024, 4096]
    # WAg[jm][p, k, gi*128+c] = WcatT[128k+p, 128*(jm+8*gi)+c]
    WAG = np.ascontiguousarray(
        WcatT.reshape(8, 128, 4, 8, 128).transpose(3, 1, 0, 2, 4)
        .reshape(8, 128, 4096))
    WRcat = np.concatenate([np.asarray(w_iouh, dtype=f32).T,
                            np.asarray(w_fh, dtype=f32).T], axis=1)  # [1024, 4096]
    # WRg[ch][p, k, c] = WRcat[128k+p, 512*ch+c]
    WRG = np.ascontiguousarray(
        WRcat.astype(f16).reshape(8, 128, 8, 512).transpose(2, 1, 0, 3)
        .reshape(8, 128, 4096))
    BT = np.empty((128, 32), dtype=f32)
    BT[:, 0:24] = b_tot.reshape(24, 128).T
    BT[:, 24:32] = b_fhx.reshape(8, 128).T
    SEL = np.zeros((128, 1024), dtype=f16)
    q = np.arange(1024)
    SEL[q // 8, q] = 1.0
    I128 = np.eye(128, dtype=f16)
    I4 = np.zeros((128, 32), dtype=f16)
    p = np.arange(128)
    I4[p, p % 32] = 1.0

    in_maps = []
    for i in range(NCORES):
        cols = _core_cols(i)
        mask = np.array([c >= 0 for c in cols])
        idx = np.array([max(c, 0) for c in cols])
        Xc = np.where(mask[None, :], X[:, idx], f32(0.0)).astype(f16)  # [1024, 1096]
        # xT[q][p, k*QW + w] = Xc[128k+p, QW*q + w]
        xT = np.ascontiguousarray(
            Xc.reshape(8, 128, 4, QW).transpose(2, 1, 0, 3).reshape(4, 128, 8 * QW))
        in_maps.append(dict(xT=xT, WAG=WAG, WRG=WRG, BT=BT, SEL=SEL,
                            I128=I128, I4=I4))
    return in_maps


def kernel(**inputs):
    nc = _build()
    in_maps = _preprocess(**inputs)
    res = run_bass_kernel_spmd(nc, in_maps, list(range(NCORES))).results
    root_h = np.asarray(res[0]["root_h"], dtype=np.float32)
    root_c = np.asarray(res[0]["root_c"], dtype=np.float32)
    return root_h, root_c


# revision 53
# speedup vs baseline: 1.1135x; 1.1135x over previous
"""ChildSumTreeLSTM (N=8192 complete 8-ary tree) on 8 TRN2 NeuronCores.

Decomposition (all tree structure is compile-time static):
- nodes 0..1023 are internal (children of p = 8p+1..8p+8), 1024..8191 leaves.
- Phase A (per core): iou_x/fx_x projections for the ~1096 node-columns this
  core owns, feature-major, fp16 matmuls on the PE (f32 PSUM accumulate).
  Leaf (h, c) states are written straight into persistent SBUF tiles
  (L4*/L3*); parent projections land in node-major SBUF tiles (nm*).
- 5 sequential rounds of internal levels: R4 (parents 585..1023, 439),
  R3 (73..584, 512), R2 (9..72, 64), R1 (1..8, 8), R0 (root).
  Each round is node-sharded across the 8 cores so that every child a core
  needs was computed locally, except: R4 results are AllGather'ed (core 0
  consumes them for R3), and R1 results are AllGather'ed (every core then
  computes the root; core 0's answer is returned).
"""
import sys
import functools

sys.path.insert(0, '/opt/trn_rl_repo')

import numpy as np
import concourse.bacc as bacc
import concourse.mybir as mybir
import concourse.tile as tile
from concourse.bass_utils import run_bass_kernel_spmd

DT = mybir.dt
AF = mybir.ActivationFunctionType
F16 = DT.float16

NCORES = 8
N = 8192
M = 1024
C4 = [54, 54, 55, 55, 55, 55, 55, 56]
S4 = [585, 639, 693, 748, 803, 858, 913, 968]
NB = [56, 64, 8, 1, 1]          # parents per round (uniform per core)
NCOLS = 1104                    # 144 parents + 448 (R4 children) + 512 (R3 children)
QW = 276                        # quarter-block width (4 quarters)


def _core_cols(i):
    # col order: [136 parents][448 R4 children][512 R3 children] so the
    # parent projections (-> nm tables) complete in quarter 0.
    cols = []
    for q in range(56):
        cols.append(S4[i] + q if q < C4[i] else -1)
    cols += [-1] * 8
    cols += [73 + 64 * i + j for j in range(64)]
    cols += [9 + 8 * i + j for j in range(8)]
    cols += [1 + i, 0] + [-1] * 6
    for pl in range(56):
        for k in range(8):
            if pl < C4[i]:
                node = 8 * (S4[i] + pl) + 1 + k
                cols.append(node if node < N else -1)
            else:
                cols.append(-1)
    for b in range(512):
        node = 585 + 512 * i + b
        cols.append(-1 if (i == 0 and b < 439) else node)
    return cols


@functools.lru_cache(maxsize=1)
def _build():
    nc = bacc.Bacc(trn_type="TRN2", target_bir_lowering=False, debug=False,
                   num_devices=NCORES)

    xT_d = nc.dram_tensor("xT", [4, 128, 8 * QW], F16, kind="ExternalInput")
    WAG_d = nc.dram_tensor("WAG", [8, 128, 4096], F16, kind="ExternalInput")
    WRG_d = nc.dram_tensor("WRG", [8, 128, 4096], F16, kind="ExternalInput")
    BT_d = nc.dram_tensor("BT", [128, 32], DT.float32, kind="ExternalInput")
    SEL_d = nc.dram_tensor("SEL", [128, 1024], F16, kind="ExternalInput")
    I_d = nc.dram_tensor("I128", [128, 128], F16, kind="ExternalInput")
    I4_d = nc.dram_tensor("I4", [128, 32], F16, kind="ExternalInput")
    I64_d = nc.dram_tensor("I64", [128, 64], F16, kind="ExternalInput")
    rh_d = nc.dram_tensor("root_h", [1, M], DT.float32, kind="ExternalOutput")
    rc_d = nc.dram_tensor("root_c", [1, M], DT.float32, kind="ExternalOutput")

    RG = [list(range(NCORES))]

    with tile.TileContext(nc) as tc:
        with (
            tc.tile_pool(name="dram", bufs=1, space="DRAM") as dram,
            tc.tile_pool(name="persist", bufs=1) as pp,
            tc.tile_pool(name="wpool", bufs=1) as wp,
            tc.tile_pool(name="leafp", bufs=1) as lp,
            tc.tile_pool(name="nmp", bufs=1) as nmp,
        ):
            spill3 = dram.tile([2, 128, 8, 512], F16)
            dum_in = dram.tile([128, 16], F16)
            dum_out = dram.tile([NCORES, 128, 16], F16, addr_space="Shared")
            ag_in = dram.tile([2, 128, 8, 56], F16)
            ag_out = dram.tile([NCORES, 2, 128, 8, 56], F16,
                               addr_space="Shared")
            agb_in = dram.tile([2, 128, 8, 1], F16)
            agb_out = dram.tile([NCORES, 2, 128, 8, 1], F16,
                                addr_space="Shared")

            I_t = pp.tile([128, 128], F16)
            I4_t = pp.tile([128, 32], F16)
            I64_t = pp.tile([128, 64], F16)
            SEL_t = pp.tile([128, 512], F16)
            BT_t = pp.tile([128, 32], DT.float32)

            # persistent R4-leaf states (feature-major: [p, jm, col]);
            # R3 leaf states spill to DRAM (readback hides under AllGather)
            L4c = lp.tile([128, 8, 448], F16, name="L4c")
            L4h = lp.tile([128, 8, 448], F16, name="L4h")

            # node-major parent projections, DVE-copied straight from the
            # Phase A transposes. nm3t keeps R3 rows at partitions 64..127
            # (consumed via I64 selectors at base 64); nm210 packs R2 rows
            # 0-7, R1 row 8, R0 row 9.
            nm4 = nmp.tile([56, 4096], F16, name="nm4")
            nm3t = nmp.tile([128, 4096], F16, name="nm3t")
            nm210 = nmp.tile([10, 4096], F16, name="nm210")
            # feature-major fx (+bias) for the 10 R2/R1/R0 parents
            pcolF = pp.tile([128, 8, 10], DT.float32, name="pcolF")

            # weight chunks: WAg[jm][p, k, gi*128+c] (Phase A), WRg[ch] (rounds)
            WAg = [wp.tile([128, 8, 512], F16, tag=f"wa{j}", bufs=1,
                           name=f"WAg{j}") for j in range(8)]
            WRg = [wp.tile([128, 8, 512], F16, tag=f"wr{j}", bufs=1,
                           name=f"WRg{j}") for j in range(8)]

            # ---------------- Phase A + leaves ----------------
            with (
                tc.tile_pool(name="xp", bufs=1) as xp,
                tc.tile_pool(name="pap", bufs=1, space="PSUM") as pap,
                tc.tile_pool(name="drp", bufs=1) as drp,
            ):
                xsq = [xp.tile([128, 8, QW], F16, tag=f"xs{q}", bufs=1,
                               name=f"xsq{q}") for q in range(4)]
                # small constants on the scalar HWDGE ring (parallel with sync)
                nc.scalar.dma_start(BT_t[:], BT_d[:])
                nc.scalar.dma_start(I_t[:], I_d[:])
                nc.scalar.dma_start(SEL_t[0:64, :], SEL_d[0:64, 0:512])
                nc.scalar.dma_start(SEL_t[64:128, :], SEL_d[0:64, 0:512])
                nc.scalar.dma_start(I4_t[:], I4_d[:])
                nc.scalar.dma_start(I64_t[:], I64_d[:])
                # dummy collective: absorb collective first-call latency and
                # launch skew while Phase A's DMA loads stream
                nc.gpsimd.collective_compute(
                    "AllGather", mybir.AluOpType.bypass, replica_groups=RG,
                    ins=[dum_in.opt()], outs=[dum_out.opt()])
                # bulk loads on the sync HWDGE ring in exact use order
                nc.sync.dma_start(
                    xsq[0][:], xT_d[0].rearrange("p (k w) -> p k w", k=8))
                for j in range(8):
                    nc.sync.dma_start(
                        WAg[j][:], WAG_d[j].rearrange("p (k c) -> p k c", k=8))
                for q in range(1, 4):
                    nc.sync.dma_start(
                        xsq[q][:], xT_d[q].rearrange("p (k w) -> p k w", k=8))
                for j in range(8):
                    nc.sync.dma_start(
                        WRg[j][:], WRG_d[j].rearrange("p (k c) -> p k c", k=8))

                def drain_ranges(b4):
                    """Leaf col ranges of quarter b4: global cols [144, 592)
                    are L4 (idx g-144), [592, 1104) spill to L3 (idx g-592).
                    Yields (is_l4, gs, ge, ls, le): dst idx range + local
                    [ls, le) range within the quarter."""
                    cb = QW * b4
                    a = cb + (144 if b4 == 0 else 0)
                    b = cb + QW
                    if a < 592:
                        e = min(b, 592)
                        yield (True, a - 144, e - 144, a - cb, e - cb)
                    if b > 592:
                        a2 = max(a, 592)
                        yield (False, a2 - 592, b - 592, a2 - cb, b - cb)

                for b4 in range(4):
                    for jm in range(8):
                        ps = {}
                        for gi in range(3):
                            ps[gi] = pap.tile([128, QW], DT.float32, tag="pa",
                                              bufs=6, name=f"pa_{jm}_{b4}_{gi}")
                        psf = None
                        if b4 == 0:
                            psf = pap.tile([128, 144], DT.float32, tag="pa",
                                           bufs=6, name=f"paf_{jm}")
                        for k in range(8):
                            for gi in range(3):
                                nc.tensor.matmul(
                                    ps[gi][:],
                                    WAg[jm][:, k, 128 * gi:128 * (gi + 1)],
                                    xsq[b4][:, k, :],
                                    start=(k == 0), stop=(k == 7))
                            if b4 == 0:
                                nc.tensor.matmul(
                                    psf[:],
                                    WAg[jm][:, k, 384:512],
                                    xsq[0][:, k, 0:144],
                                    start=(k == 0), stop=(k == 7))
                        # leaf elementwise drains -> straight into SBUF state
                        ls0 = 144 if b4 == 0 else 0
                        si = drp.tile([128, QW], F16, tag="dr", bufs=8,
                                      name=f"si_{jm}_{b4}")
                        tu = drp.tile([128, QW], F16, tag="dr", bufs=8,
                                      name=f"tu_{jm}_{b4}")
                        nc.scalar.activation(si[:, ls0:QW], ps[0][:, ls0:QW],
                                             AF.Sigmoid,
                                             bias=BT_t[:, jm:jm + 1])
                        nc.scalar.activation(tu[:, ls0:QW], ps[2][:, ls0:QW],
                                             AF.Tanh,
                                             bias=BT_t[:, jm + 16:jm + 17])
                        ct3 = None
                        for is4, gs, ge, ls, le in drain_ranges(b4):
                            if is4:
                                nc.vector.tensor_mul(L4c[:, jm, gs:ge],
                                                     si[:, ls:le], tu[:, ls:le])
                            else:
                                ct3 = drp.tile([128, QW], F16, tag="c3",
                                               bufs=8, name=f"ct3_{jm}_{b4}")
                                nc.vector.tensor_mul(ct3[:, ls:le],
                                                     si[:, ls:le], tu[:, ls:le])
                                nc.scalar.dma_start(spill3[0, :, jm, gs:ge],
                                                    ct3[:, ls:le])
                        so = drp.tile([128, QW], F16, tag="dr", bufs=8,
                                      name=f"so_{jm}_{b4}")
                        nc.scalar.activation(so[:, ls0:QW], ps[1][:, ls0:QW],
                                             AF.Sigmoid,
                                             bias=BT_t[:, jm + 8:jm + 9])
                        tanc = drp.tile([128, QW], F16, tag="dr", bufs=8,
                                        name=f"tanc_{jm}_{b4}")
                        for is4, gs, ge, ls, le in drain_ranges(b4):
                            if is4:
                                nc.scalar.activation(tanc[:, ls:le],
                                                     L4c[:, jm, gs:ge], AF.Tanh)
                                nc.vector.tensor_mul(L4h[:, jm, gs:ge],
                                                     so[:, ls:le], tanc[:, ls:le])
                            else:
                                nc.scalar.activation(tanc[:, ls:le],
                                                     ct3[:, ls:le], AF.Tanh)
                                ht3 = drp.tile([128, QW], F16, tag="c3",
                                               bufs=8, name=f"ht3_{jm}_{b4}")
                                nc.vector.tensor_mul(ht3[:, ls:le],
                                                     so[:, ls:le], tanc[:, ls:le])
                                nc.sync.dma_start(spill3[1, :, jm, gs:ge],
                                                  ht3[:, ls:le])
                        # parent drains + transpose to node-major (cols
                        # 0..144 of quarter 0; [56 R4][8 pad][64 R3][8 R2]
                        # [R1][R0][6 pad]) -> DVE copies straight into the
                        # SBUF nm tables (R3 rows stay at partitions 64..127)
                        if b4 == 0:
                            for gi in range(4):
                                j = jm + 8 * gi
                                bcol = j if gi < 3 else 24 + jm
                                src = (ps[gi][:, 0:144] if gi < 3
                                       else psf[:, 0:144])
                                fm = drp.tile([128, 144], F16, tag="fm",
                                              bufs=4, name=f"fm_{jm}_{gi}")
                                nc.scalar.activation(
                                    fm[:, 0:144], src, AF.Identity,
                                    bias=BT_t[:, bcol:bcol + 1])
                                cs = slice(128 * j, 128 * (j + 1))
                                tp = pap.tile([128, 144], F16,
                                              tag="tp", bufs=2,
                                              name=f"tp_{jm}_{gi}_0")
                                nc.tensor.transpose(
                                    tp[0:128, 0:128], fm[:, 0:128], I_t[:, :])
                                nc.vector.tensor_copy(nm4[0:56, cs],
                                                      tp[0:56, 0:128])
                                nc.vector.tensor_copy(nm3t[64:128, cs],
                                                      tp[64:128, 0:128])
                                tp2 = pap.tile([128, 144], F16,
                                               tag="tp", bufs=2,
                                               name=f"tp_{jm}_{gi}_1")
                                nc.tensor.transpose(
                                    tp2[0:16, 0:128], fm[:, 128:144],
                                    I_t[:, :])
                                nc.vector.tensor_copy(nm210[0:10, cs],
                                                      tp2[0:10, 0:128])
                                if gi == 3:
                                    nc.vector.tensor_copy(
                                        pcolF[:, jm, 0:10], fm[:, 128:138])
                # zero the single real pad-child column (local col 447:
                # node 1023's 8th child on core 7; harmless on other cores)
                nc.vector.memset(L4c[:, :, 447:448], 0.0)
                nc.vector.memset(L4h[:, :, 447:448], 0.0)

            # ---------------- Rounds ----------------
            import os as _os
            _SKIP_ROUNDS = bool(_os.environ.get('PHASE_A_ONLY'))
            _UPTO = int(_os.environ.get('ROUNDS_UPTO', '99'))
            if _SKIP_ROUNDS:
                dum = pp.tile([128, 8], DT.float32, name="dum")
                nc.vector.memset(dum[:], 0.0)
                nc.sync.dma_start(
                    rc_d[0, :].rearrange("(m p) -> p m", p=128), dum[:])
                nc.sync.dma_start(
                    rh_d[0, :].rearrange("(m p) -> p m", p=128), dum[:])
            if not _SKIP_ROUNDS:
              with (
                  tc.tile_pool(name="rps", bufs=1, space="PSUM") as rps,
                  tc.tile_pool(name="chp", bufs=1) as chp,
                  tc.tile_pool(name="rwp", bufs=1) as rwp,
                  tc.tile_pool(name="sink", bufs=1) as sink,
              ):
                  ch3c = chp.tile([128, 8, 512], F16, name="ch3c")
                  ch3h = chp.tile([128, 8, 512], F16, name="ch3h")
                  c3_c = sink.tile([128, 8, 64], F16)
                  c3_h = sink.tile([128, 8, 64], F16)
                  c2_c = sink.tile([128, 8, 8], F16)
                  c2_h = sink.tile([128, 8, 8], F16)
                  c1_c = sink.tile([128, 8, 8], F16)
                  c1_h = sink.tile([128, 8, 8], F16)
                  st4_c = sink.tile([128, 8, 56], F16)
                  st4_h = sink.tile([128, 8, 56], F16)
                  st1_c = sink.tile([128, 8, 1], F16)
                  st1_h = sink.tile([128, 8, 1], F16)
                  c1raw = sink.tile([8, 2, 128, 8], F16)
                  rootc_sb = sink.tile([128, 8], F16)
                  rooth_sb = sink.tile([128, 8], F16)
                  rootc_f32 = sink.tile([128, 8], DT.float32)
                  rooth_f32 = sink.tile([128, 8], DT.float32)

                  # (tile, selector-row offset, selector rows, partition base)
                  NM = [(nm4, 0, 56, 0), (nm3t, 0, 64, 64), (nm210, 0, 10, 0),
                        (nm210, 8, 10, 0), (nm210, 9, 10, 0)]

                  def group8_sum(prod_ap, out_ap, nb, rn, jm):
                      """out[p, n] = sum_k prod[p, 8n + k]."""
                      a = prod_ap.rearrange("p (n k) -> p n k", k=8)
                      l1 = rwp.tile([128, 256], F16, tag="lvl1", bufs=2,
                                    name=f"l1_{rn}_{jm}")
                      l1v = l1[:, 0:nb * 4].rearrange("p (n k) -> p n k", k=4)
                      nc.vector.tensor_add(l1v, a[:, :, 0:4], a[:, :, 4:8])
                      l2 = rwp.tile([128, 128], F16, tag="lvl2", bufs=2,
                                    name=f"l2_{rn}_{jm}")
                      l2v = l2[:, 0:nb * 2].rearrange("p (n k) -> p n k", k=2)
                      nc.vector.tensor_add(l2v, l1v[:, :, 0:2], l1v[:, :, 2:4])
                      # out[p, n] = l2[p, 2n] + l2[p, 2n+1]  (stride-2 views)
                      e0 = l2v[:, :, 0:1].rearrange("p n k -> p (n k)")
                      e1 = l2v[:, :, 1:2].rearrange("p n k -> p (n k)")
                      nc.vector.tensor_add(out_ap, e0, e1)

                  def iou_small(rn, nb, nm_t, r0, nrows, nmb, csumT):
                      """Col-tiled iou for nb<=8: each ch gets its own PSUM
                      bank; 4-way array col-group concurrency."""
                      ipsc = [rps.tile([128, 512], DT.float32, tag="iou",
                                       bufs=6, name=f"ipsS_{rn}_{c}")
                              for c in range(6)]
                      for k in range(8):
                          for ch in range(6):
                              g = 32 * (ch % 4)
                              nc.tensor.matmul(
                                  ipsc[ch][g:g + nb, :],
                                  csumT[:, k, 0:nb],
                                  WRg[ch][:, k, :],
                                  start=(k == 0), stop=False,
                                  tile_position=(0, g))
                      for ch in range(6):
                          g = 32 * (ch % 4)
                          nc.tensor.matmul(
                              ipsc[ch][g:g + nb, :],
                              I_t[0:nrows, r0:r0 + nb],
                              nm_t[0:nrows, 512 * ch:512 * (ch + 1)],
                              start=False, stop=True,
                              tile_position=(0, g))
                      return ipsc

                  def iou_full(rn, nb, nm_t, r0, nrows, nmb, csumT):
                      """Node-major iou, one PSUM bank per ch."""
                      ipsc = [rps.tile([64, 512], DT.float32, tag="iou",
                                       bufs=6, name=f"ips_{rn}_{c}")
                              for c in range(6)]
                      for k in range(8):
                          for ch in range(6):
                              nc.tensor.matmul(
                                  ipsc[ch][0:nb, :],
                                  csumT[:, k, 0:nb],
                                  WRg[ch][:, k, :],
                                  start=(k == 0), stop=False)
                      idsel = I_t if nmb == 0 else I64_t
                      for ch in range(6):
                          nc.tensor.matmul(
                              ipsc[ch][0:nb, :],
                              idsel[nmb:nmb + nrows, r0:r0 + nb],
                              nm_t[nmb:nmb + nrows, 512 * ch:512 * (ch + 1)],
                              start=False, stop=True,
                              tile_position=(nmb, 0))
                      return ipsc

                  def run_round(rn, get_chC, get_chH, out_c, out_h):
                      nb = NB[rn]
                      nm_t, r0, nrows, nmb = NM[rn]
                      w8 = 8 * nb
                      small = nb <= 8
                      # 1. csum (feature-major)
                      csumT = rwp.tile([128, 8, 64], F16, tag="csum",
                                       bufs=1, name=f"csum_{rn}")
                      for m in range(8):
                          group8_sum(get_chC(m), csumT[:, m, 0:nb], nb, rn, m)
                      # 2. iou
                      if small:
                          ipsc = iou_small(rn, nb, nm_t, r0, nrows, nmb, csumT)
                      else:
                          ipsc = iou_full(rn, nb, nm_t, r0, nrows, nmb, csumT)
                      # 3-5. f gates (feature-major), prod, fc
                      fcT = rwp.tile([128, 8, 64], F16, tag="fcT", bufs=1,
                                     name=f"fcT_{rn}")
                      use_bias_fx = nb == 1
                      for j in range(8):
                          fps = rps.tile([128, 512], DT.float32, tag="fp", bufs=2,
                                         name=f"fps_{rn}_{j}")
                          for k in range(8):
                              nc.tensor.matmul(
                                  fps[:, 0:w8],
                                  WRg[6 + j // 4][:, k,
                                                  128 * (j % 4):128 * (j % 4 + 1)],
                                  get_chC(k)[:, 0:w8],
                                  start=(k == 0),
                                  stop=(use_bias_fx and k == 7))
                          if not use_bias_fx:
                              nc.tensor.matmul(
                                  fps[:, 0:w8],
                                  nm_t[nmb + r0:nmb + r0 + nb,
                                       3072 + 128 * j:3072 + 128 * (j + 1)],
                                  SEL_t[nmb:nmb + nb, 0:w8],
                                  start=False, stop=True,
                                  tile_position=(nmb, 0))
                          fsb = rwp.tile([128, 512], F16, tag="fsb", bufs=2,
                                         name=f"fsb_{rn}_{j}")
                          if use_bias_fx:
                              # fx (+b) add via per-partition activation bias
                              nc.scalar.activation(
                                  fsb[:, 0:w8], fps[:, 0:w8], AF.Sigmoid,
                                  bias=pcolF[:, j, r0:r0 + 1])
                          else:
                              nc.scalar.activation(fsb[:, 0:w8], fps[:, 0:w8],
                                                   AF.Sigmoid)
                          prod = rwp.tile([128, 512], F16, tag="fsb", bufs=2,
                                          name=f"prod_{rn}_{j}")
                          nc.vector.tensor_mul(prod[:, 0:w8], fsb[:, 0:w8],
                                               get_chH(j)[:, 0:w8])
                          group8_sum(prod[:, 0:w8], fcT[:, j, 0:nb], nb, rn, 100 + j)
                      # 6. gates from iou psum
                      tw = max(2, nb)
                      if small:
                          # ch c lives at partitions 32*(c%4).. of its bank
                          gio = rwp.tile([128, 512], F16, tag="g", bufs=3,
                                         name=f"gio_{rn}")
                          for c in range(4):
                              g = 32 * c
                              nc.scalar.activation(gio[g:g + nb, :],
                                                   ipsc[c][g:g + nb, :],
                                                   AF.Sigmoid)
                          gu = rwp.tile([64, 512], F16, tag="g", bufs=3,
                                        name=f"gu_{rn}")
                          for c in range(2):
                              g = 32 * c
                              nc.scalar.activation(gu[g:g + nb, :],
                                                   ipsc[4 + c][g:g + nb, :],
                                                   AF.Tanh)
                          p1 = rwp.tile([64, 512], F16, tag="g", bufs=3,
                                        name=f"p1_{rn}")
                          nc.vector.tensor_mul(p1[0:64, :], gio[0:64, :],
                                               gu[0:64, :])

                          def tsrc(which, m):
                              # (tile, row base) of feature chunk m
                              if which == 'p1':
                                  return p1, 32 * (m // 4)
                              return gio, 64 + 32 * (m // 4)
                      else:
                          si = rwp.tile([64, 1024], F16, tag="g", bufs=3,
                                        name=f"si_{rn}")
                          tu = rwp.tile([64, 1024], F16, tag="g", bufs=3,
                                        name=f"tu_{rn}")
                          for c in range(2):
                              nc.scalar.activation(si[0:nb, 512 * c:512 * (c + 1)],
                                                   ipsc[c][0:nb, :], AF.Sigmoid)
                              nc.scalar.activation(tu[0:nb, 512 * c:512 * (c + 1)],
                                                   ipsc[4 + c][0:nb, :], AF.Tanh)
                          p1 = rwp.tile([64, 1024], F16, tag="g", bufs=3,
                                        name=f"p1_{rn}")
                          nc.vector.tensor_mul(p1[0:nb, :], si[0:nb, :],
                                               tu[0:nb, :])
                          so = rwp.tile([64, 1024], F16, tag="g", bufs=3,
                                        name=f"so_{rn}")
                          for c in range(2):
                              nc.scalar.activation(so[0:nb, 512 * c:512 * (c + 1)],
                                                   ipsc[2 + c][0:nb, :],
                                                   AF.Sigmoid)

                          def tsrc(which, m):
                              if which == 'p1':
                                  return p1, 0
                              return so, 0
                      # 7-8. transpose to feature-major, combine
                      for m in range(8):
                          if small:
                              mm = 128 * (m % 4)
                          else:
                              mm = 128 * m
                          t1, b1 = tsrc('p1', m)
                          idm = I_t if b1 == 0 else I4_t
                          tp1 = rps.tile([128, 64], F16, tag="fp", bufs=2,
                                         name=f"tp1_{rn}_{m}")
                          nc.tensor.transpose(tp1[:, 0:tw],
                                              t1[b1:b1 + nb, mm:mm + 128],
                                              idm[b1:b1 + nb, 0:tw],
                                              tile_position=(b1, 0))
                          cm = out_c(m)
                          nc.vector.tensor_add(cm, tp1[:, 0:nb], fcT[:, m, 0:nb])
                          t2, b2 = tsrc('so', m)
                          idm = I_t if b2 == 0 else I4_t
                          tso = rps.tile([128, 64], F16, tag="fp", bufs=2,
                                         name=f"tso_{rn}_{m}")
                          nc.tensor.transpose(tso[:, 0:tw],
                                              t2[b2:b2 + nb, mm:mm + 128],
                                              idm[b2:b2 + nb, 0:tw],
                                              tile_position=(b2, 0))
                          tanc = rwp.tile([128, 64], F16, tag="tanc", bufs=2,
                                          name=f"tanc_{rn}_{m}")
                          nc.scalar.activation(tanc[:, 0:nb], cm, AF.Tanh)
                          nc.vector.tensor_mul(out_h(m), tso[:, 0:nb],
                                               tanc[:, 0:nb])

                  # ---- R4 ----
                  if _UPTO >= 1:
                    run_round(0,
                            lambda m: L4c[:, m, 0:448],
                            lambda m: L4h[:, m, 0:448],
                            lambda m: st4_c[:, m, 0:56],
                            lambda m: st4_h[:, m, 0:56])
                  if _UPTO >= 2:
                    nc.sync.dma_start(ag_in[0], st4_c[:])
                    nc.sync.dma_start(ag_in[1], st4_h[:])
                    nc.gpsimd.collective_compute(
                        "AllGather", mybir.AluOpType.bypass, replica_groups=RG,
                        ins=[ag_in.opt()], outs=[ag_out.opt()])
                  # ---- R3 ----
                  if _UPTO >= 3:
                    nc.sync.dma_start(ch3c[:], spill3[0])
                    nc.scalar.dma_start(ch3h[:], spill3[1])
                    pid = nc.gpsimd.partition_id()
                    with tc.If(pid == 0):
                        for r in range(NCORES):
                            off = S4[r] - 585
                            nc.gpsimd.dma_start(
                                ch3c[:, :, off:off + C4[r]],
                                ag_out[r, 0, :, :, 0:C4[r]])
                            nc.gpsimd.dma_start(
                                ch3h[:, :, off:off + C4[r]],
                                ag_out[r, 1, :, :, 0:C4[r]])
                    run_round(1,
                            lambda m: ch3c[:, m, 0:512],
                            lambda m: ch3h[:, m, 0:512],
                            lambda m: c3_c[:, m, 0:64],
                            lambda m: c3_h[:, m, 0:64])
                  # ---- R2 ----
                  if _UPTO >= 3:
                    run_round(2,
                            lambda m: c3_c[:, m, :], lambda m: c3_h[:, m, :],
                            lambda m: c2_c[:, m, 0:8],
                            lambda m: c2_h[:, m, 0:8])
                  # ---- R1 ----
                  if _UPTO >= 4:
                    run_round(3,
                            lambda m: c2_c[:, m, :], lambda m: c2_h[:, m, :],
                            lambda m: st1_c[:, m, 0:1],
                            lambda m: st1_h[:, m, 0:1])
                  if _UPTO >= 5:
                    nc.sync.dma_start(agb_in[0], st1_c[:])
                    nc.sync.dma_start(agb_in[1], st1_h[:])
                    nc.gpsimd.collective_compute(
                        "AllGather", mybir.AluOpType.bypass, replica_groups=RG,
                        ins=[agb_in.opt()], outs=[agb_out.opt()])
                    nc.sync.dma_start(
                        c1raw[:], agb_out.rearrange("r s p k one -> r s p (k one)"))
                    for state in range(2):
                        dst = c1_c if state == 0 else c1_h
                        for k in range(8):
                            tpk = rps.tile([128, 8], F16, tag="fp", bufs=2,
                                           name=f"tpk_{state}_{k}")
                            nc.tensor.transpose(tpk[:, 0:8],
                                                c1raw[0:8, state, :, k],
                                                I_t[0:8, 0:8])
                            nc.vector.tensor_copy(dst[:, k, 0:8], tpk[:, 0:8])
                  # ---- R0 ----
                  if _UPTO >= 6:
                    run_round(4,
                            lambda m: c1_c[:, m, :], lambda m: c1_h[:, m, :],
                            lambda m: rootc_sb[:, m:m + 1],
                            lambda m: rooth_sb[:, m:m + 1])
                  if _UPTO < 6:
                      nc.vector.memset(rootc_sb[:], 0.0)
                      nc.vector.memset(rooth_sb[:], 0.0)
                  nc.vector.tensor_copy(rootc_f32[:], rootc_sb[:])
                  nc.vector.tensor_copy(rooth_f32[:], rooth_sb[:])
                  nc.sync.dma_start(
                      rc_d[0, :].rearrange("(m p) -> p m", p=128), rootc_f32[:])
                  nc.sync.dma_start(
                      rh_d[0, :].rearrange("(m p) -> p m", p=128), rooth_f32[:])

    nc.compile()
    return nc


def _preprocess(inputs, children, w_ioux, b_ioux, w_iouh, b_iouh,
                w_fx, b_fx, w_fh, b_fh):
    f32 = np.float32
    f16 = np.float16
    inputs = np.ascontiguousarray(inputs, dtype=f32)
    b_tot = (np.asarray(b_ioux) + np.asarray(b_iouh)).astype(f32)
    b_fhx = (np.asarray(b_fx) + np.asarray(b_fh)).astype(f32)

    X = inputs.T                                           # [1024, 8192]
    Wcat = np.concatenate([np.asarray(w_ioux, dtype=f32),
                           np.asarray(w_fx, dtype=f32)], axis=0)   # [4096, 1024]
    WcatT = Wcat.T.astype(f16)                             # [1024, 4096]
    # WAg[jm][p, k, gi*128+c] = WcatT[128k+p, 128*(jm+8*gi)+c]
    WAG = np.ascontiguousarray(
        WcatT.reshape(8, 128, 4, 8, 128).transpose(3, 1, 0, 2, 4)
        .reshape(8, 128, 4096))
    WRcat = np.concatenate([np.asarray(w_iouh, dtype=f32).T,
                            np.asarray(w_fh, dtype=f32).T], axis=1)  # [1024, 4096]
    # WRg[ch][p, k, c] = WRcat[128k+p, 512*ch+c]
    WRG = np.ascontiguousarray(
        WRcat.astype(f16).reshape(8, 128, 8, 512).transpose(2, 1, 0, 3)
        .reshape(8, 128, 4096))
    BT = np.empty((128, 32), dtype=f32)
    BT[:, 0:24] = b_tot.reshape(24, 128).T
    BT[:, 24:32] = b_fhx.reshape(8, 128).T
    SEL = np.zeros((128, 1024), dtype=f16)
    q = np.arange(1024)
    SEL[q // 8, q] = 1.0
    I128 = np.eye(128, dtype=f16)
    I4 = np.zeros((128, 32), dtype=f16)
    p = np.arange(128)
    I4[p, p % 32] = 1.0
    I64 = np.zeros((128, 64), dtype=f16)
    I64[p, p % 64] = 1.0

    in_maps = []
    for i in range(NCORES):
        cols = _core_cols(i)
        mask = np.array([c >= 0 for c in cols])
        idx = np.array([max(c, 0) for c in cols])
        Xc = np.where(mask[None, :], X[:, idx], f32(0.0)).astype(f16)  # [1024, 1096]
        # xT[q][p, k*QW + w] = Xc[128k+p, QW*q + w]
        xT = np.ascontiguousarray(
            Xc.reshape(8, 128, 4, QW).transpose(2, 1, 0, 3).reshape(4, 128, 8 * QW))
        in_maps.append(dict(xT=xT, WAG=WAG, WRG=WRG, BT=BT, SEL=SEL,
                            I128=I128, I4=I4, I64=I64))
    return in_maps


def kernel(**inputs):
    nc = _build()
    in_maps = _preprocess(**inputs)
    res = run_bass_kernel_spmd(nc, in_maps, list(range(NCORES))).results
    root_h = np.asarray(res[0]["root_h"], dtype=np.float32)
    root_c = np.asarray(res[0]["root_c"], dtype=np.float32)
    return root_h, root_c
